# revision 1
# baseline (speedup 1.0000x reference)
"""DFine MultiScale Deformable Attention — Trainium2 Bass kernel, v2.

Data-parallel over batch (4 per core x 8 cores). Per batch:
  1. SWDGE cast-DMA: value f32 HBM -> V1 bf16 DRAM.
  2. Per 128-ch half g: X-bar DMA transpose V1[:,128g:] -> SBUF table
     Te [128, 8448] bf16; shifted copy builds To (odd pixel pairs) so
     T = [Te | To], and ap_gather(d=2) at idx = m + 4224*par fetches the
     (x, x+1) pixel pair for any parity.
  3. Frontend in [96=(h,p), 336=q] layout: offsets/attn via PE matmuls,
     softmax, bilinear slot weights (wA, wB per x-pair slot), pair idx.
  4. Idx wrap via DRAM roundtrip into [128=(hl,dh,a), (p,y,k)] streams.
  5. ap_gather per (half, p-chunk): G [128, 3648, 2] bf16.
  6. PE broadcasts weights (selp matmuls) -> ACT cast to bf16 -> DVE
     multiply-accumulate over points -> y/s reduction -> output transpose.
"""

import numpy as np
import ml_dtypes

import concourse.bass as bass
import concourse.tile as tile
from concourse import bacc, mybir, library_config
from concourse.bass_utils import run_bass_kernel_spmd

F32 = mybir.dt.float32
BF16 = mybir.dt.bfloat16
I16 = mybir.dt.int16

B, LQ, DM, NH, HD = 32, 300, 256, 8, 32
NP_TOT = 12
LVL_W = [80, 40, 20]
LVL_BASE = [0, 6400, 8000]
S = 8400
S_PAD = 8448
NPAIR = S_PAD // 2                 # 4224
PB = 2 * S_PAD                     # patch region offset (elems)
NPATCH = 2048                      # L1+L2 patch entries (1600+400, padded)
S12 = 6400                         # L12 pixel base
N_CORES = 8
BPC = B // N_CORES
SHIFT = 64.0
CAST_BIAS = SHIFT - 1.0
REPEAT = 1

QP, NSLOT, Q336 = 112, 3, 336
KG = 19                            # global k blocks: q = 16k + a, q < 304
Q304 = 304
NCOL = 8 * KG                      # 152 idx cols per stream (L0: p4*y2, L12: p8)
KCH = ((0, 7), (7, 14), (14, KG))  # k-chunks (= query slots)

ALL_STAGES = ("table", "front", "wrap", "gather", "reduce")


def _wl(p):
    return float(LVL_W[p // 4])


def _base(p):
    return float(LVL_BASE[p // 4])


def make_consts():
    c = {}
    c["ident"] = np.eye(128, dtype=np.float32)
    selp = np.zeros((96, 2 * 12 * 128), np.float32)
    for g in range(2):
        for p in range(12):
            for hl in range(4):
                for d in range(32):
                    selp[(4 * g + hl) * 12 + p,
                         (g * 12 + p) * 128 + hl * 32 + d] = 1.0
    c["selp"] = selp.astype(ml_dtypes.bfloat16)
    sel8 = np.zeros((96, 8), np.float32)
    rep8 = np.zeros((8, 96), np.float32)
    for h in range(8):
        for p in range(12):
            sel8[h * 12 + p, h] = 1.0
            rep8[h, h * 12 + p] = 1.0
    c["sel8"] = sel8
    c["rep8"] = rep8
    selr = np.zeros((4, 4 * 96), np.float32)
    for hp in range(96):
        p = hp % 12
        w = _wl(p)
        selr[0, 0 * 96 + hp] = w
        selr[1, 1 * 96 + hp] = w
        selr[2, 2 * 96 + hp] = 0.125 * w
        selr[3, 3 * 96 + hp] = 0.125 * w
    c["selr"] = selr
    c["ones1"] = np.ones((1, Q336), np.float32)
    cv = np.zeros((96, 8), np.float32)
    for hp in range(96):
        p = hp % 12
        w, base = _wl(p), _base(p)
        cv[hp, 0] = w - 1.0 + SHIFT                   # XMAX (x0 or W-1+S)
        cv[hp, 1] = w - 2.0 + SHIFT                   # XM2 (xb / x1 clip)
        cv[hp, 2] = w                                 # W
        cv[hp, 3] = base - SHIFT * w - SHIFT          # lin const y0
        cv[hp, 4] = base - float(S12) - SHIFT * w - SHIFT  # patch idx const
        cv[hp, 5] = base - (SHIFT - 1.0) * w - SHIFT  # lin const y1
        cv[hp, 6] = 0.0
    c["cv"] = cv
    return c


def emit(nc, repeat=None, stages=ALL_STAGES):
    if repeat is None:
        repeat = REPEAT
    stages = set(stages)

    value = nc.dram_tensor("value", [BPC, S, DM], F32, kind="ExternalInput").ap()
    query = nc.dram_tensor("query", [BPC, LQ, DM], F32, kind="ExternalInput").ap()
    refp = nc.dram_tensor("refp", [BPC, LQ, 4], F32, kind="ExternalInput").ap()
    woff = nc.dram_tensor("woff", [DM, 192], F32, kind="ExternalInput").ap()
    wattn = nc.dram_tensor("wattn", [DM, 96], F32, kind="ExternalInput").ap()
    boff = nc.dram_tensor("boff", [1, 192], F32, kind="ExternalInput").ap()
    battn = nc.dram_tensor("battn", [1, 96], F32, kind="ExternalInput").ap()
    ident_d = nc.dram_tensor("ident", [128, 128], F32, kind="ExternalInput").ap()
    selp_d = nc.dram_tensor("selp", [96, 24 * 128], BF16, kind="ExternalInput").ap()
    sel8_d = nc.dram_tensor("sel8", [96, 8], F32, kind="ExternalInput").ap()
    rep8_d = nc.dram_tensor("rep8", [8, 96], F32, kind="ExternalInput").ap()
    selr_d = nc.dram_tensor("selr", [4, 4 * 96], F32, kind="ExternalInput").ap()
    ones1_d = nc.dram_tensor("ones1", [1, Q336], F32, kind="ExternalInput").ap()
    cv_d = nc.dram_tensor("cv", [96, 8], F32, kind="ExternalInput").ap()
    out_d = nc.dram_tensor("out", [BPC, LQ, DM], F32, kind="ExternalOutput").ap()
    v1_d = nc.dram_tensor("v1", [BPC, S, DM], BF16, kind="Internal").ap()
    linq_d = nc.dram_tensor("linq", [BPC, QP, NSLOT * 8 * 16], I16,
                            kind="Internal").ap()

    MUL, ADD, SUB, MAX, MIN, EQ = (
        mybir.AluOpType.mult, mybir.AluOpType.add, mybir.AluOpType.subtract,
        mybir.AluOpType.max, mybir.AluOpType.min, mybir.AluOpType.is_equal)
    EXP = mybir.ActivationFunctionType.Exp

    with tile.TileContext(nc) as tc:
        import contextlib
        ctx = contextlib.ExitStack()
        with ctx:
            cpool = ctx.enter_context(tc.tile_pool(name="consts", bufs=1))
            tpool = ctx.enter_context(tc.tile_pool(name="tables", bufs=1))
            gpool = ctx.enter_context(tc.tile_pool(name="gath", bufs=3))
            fpool = ctx.enter_context(tc.tile_pool(name="front", bufs=18))
            f16pool = ctx.enter_context(tc.tile_pool(name="fi16", bufs=1))
            wpool = ctx.enter_context(tc.tile_pool(name="w8", bufs=2))
            bpool = ctx.enter_context(tc.tile_pool(name="wpb", bufs=2))
            mpool = ctx.enter_context(tc.tile_pool(name="mm", bufs=2))
            apool = ctx.enter_context(tc.tile_pool(name="accm", bufs=2))
            ypool = ctx.enter_context(tc.tile_pool(name="ysp", bufs=1))
            opool = ctx.enter_context(tc.tile_pool(name="outsb", bufs=2))
            ipool = ctx.enter_context(tc.tile_pool(name="idx", bufs=2))
            qpool = ctx.enter_context(tc.tile_pool(name="qt", bufs=1))
            psS = ctx.enter_context(tc.tile_pool(name="psS", bufs=2, space="PSUM"))
            psW = ctx.enter_context(tc.tile_pool(name="psW", bufs=3, space="PSUM"))
            psF = ctx.enter_context(tc.tile_pool(name="psF", bufs=2, space="PSUM"))

            nc.gpsimd.load_library(library_config.ap_gather)

            def ld(dst, src):
                nc.sync.dma_start(dst, src)

            ident = cpool.tile([128, 128], F32, name="ident")
            ld(ident[:], ident_d)
            selp = cpool.tile([96, 24 * 128], BF16, name="selp")
            ld(selp[:], selp_d)
            sel8 = cpool.tile([96, 8], F32, name="sel8")
            ld(sel8[:], sel8_d)
            rep8 = cpool.tile([8, 96], F32, name="rep8")
            ld(rep8[:], rep8_d)
            selr = cpool.tile([4, 4 * 96], F32, name="selr")
            ld(selr[:], selr_d)
            ones1 = cpool.tile([1, Q336], F32, name="ones1")
            ld(ones1[:], ones1_d)
            cv = cpool.tile([96, 8], F32, name="cv")
            ld(cv[:], cv_d)
            woff_sb = cpool.tile([128, 2, 192], F32, name="woff_sb")
            ld(woff_sb[:], woff.rearrange("(kt p) m -> p kt m", p=128))
            wattn_sb = cpool.tile([128, 2, 96], F32, name="wattn_sb")
            ld(wattn_sb[:], wattn.rearrange("(kt p) m -> p kt m", p=128))
            boff_sb = cpool.tile([1, 192], F32, name="boff_sb")
            ld(boff_sb[:], boff)
            battn_sb = cpool.tile([1, 96], F32, name="battn_sb")
            ld(battn_sb[:], battn)

            def cvs(k):
                return cv[:, k:k + 1]

            def fs(nm):
                return fpool.tile([96, Q336], F32, name=nm, tag="fs")

            if "table" in stages:
                for b in range(BPC):
                    nc.gpsimd.dma_start(v1_d[b], value[b])

            for b4 in range(BPC * repeat):
                b = b4 % BPC
                # ==== 1+2. table build ===================================
                Th = None
                if "table" in stages:
                    Th = [tpool.tile([128, PB + 4 * NPATCH], BF16, name=f"T{g}",
                                     tag=f"T{g}") for g in range(2)]
                    for g in range(2):
                        nc.scalar.dma_start(Th[g][:, 0:S],
                                            v1_d[b][:, 128 * g:128 * (g + 1)],
                                            transpose=True)
                        nc.vector.memset(Th[g][:, S:S_PAD], 0.0)
                        nc.vector.memset(Th[g][:, 2 * S_PAD - 2:2 * S_PAD], 0.0)
                        cp = nc.vector.tensor_copy if g == 0 else nc.scalar.copy
                        cp(Th[g][:, S_PAD:2 * S_PAD - 2], Th[g][:, 1:S_PAD - 1])
                        # L12 patch: P[4*s'+c] = v[S12+s'+d(c)], d=(0,1,W,W+1)
                        for (lo, hi, w) in ((0, 1600, 40), (1600, 2000, 20)):
                            for c, dlt in enumerate((0, 1, w, w + 1)):
                                dst = Th[g][:, PB + 4 * lo + c:
                                            PB + 4 * hi + c].rearrange(
                                    "r (n f) -> r n f", f=4)[:, :, 0]
                                cp2 = (nc.vector.tensor_copy if (c % 2 == g)
                                       else nc.scalar.copy)
                                cp2(dst, Th[g][:, S12 + lo + dlt:
                                               S12 + hi + dlt])

                # ==== 3. frontend ========================================
                if "front" not in stages:
                    continue
                qsb = qpool.tile([QP, NSLOT, DM], F32, name="qsb", tag="qsb")
                nc.vector.memset(qsb[64:112, 2, :], 0.0)
                nc.sync.dma_start(
                    qsb[:, 0:2, :],
                    query[b][0:224].rearrange("(s r) c -> r s c", r=QP))
                nc.sync.dma_start(qsb[0:76, 2, :], query[b][224:300, :])
                refsb = qpool.tile([QP, NSLOT, 4], F32, name="refsb", tag="refsb")
                nc.vector.memset(refsb[64:112, 2, :], 0.0)
                nc.sync.dma_start(
                    refsb[:, 0:2, :],
                    refp[b][0:224].rearrange("(s r) c -> r s c", r=QP))
                nc.sync.dma_start(refsb[0:76, 2, :], refp[b][224:300, :])

                qT = [qpool.tile([128, Q336], F32, name=f"qT{kt}", tag=f"qT{kt}")
                      for kt in range(2)]
                for slot in range(NSLOT):
                    for kt in range(2):
                        pt = psS.tile([128, 128], F32, name="pss", tag="pss")
                        nc.tensor.transpose(
                            pt[:, 0:QP], qsb[:, slot, kt * 128:(kt + 1) * 128],
                            ident[0:QP, 0:QP])
                        nc.scalar.copy(qT[kt][:, slot * QP:(slot + 1) * QP],
                                       pt[:, 0:QP])
                refT = fs("refT")
                for slot in range(NSLOT):
                    pt = psS.tile([128, 128], F32, name="pss", tag="pss")
                    nc.tensor.transpose(pt[0:4, 0:QP], refsb[:, slot, :],
                                        ident[0:QP, 0:QP])
                    nc.scalar.copy(refT[0:4, slot * QP:(slot + 1) * QP],
                                   pt[0:4, 0:QP])

                refb = []
                for m in range(4):
                    ps = psF.tile([96, Q336], F32, name="psf", tag="psf")
                    nc.tensor.matmul(ps[:], selr[:, m * 96:(m + 1) * 96],
                                     refT[0:4, :])
                    sb = fs(f"refb{m}")
                    nc.scalar.copy(sb[:], ps[:])
                    refb.append(sb)
                rxw, ryw, rwc, rhc = refb

                def head_mm(w_sb, bias_sb, cols, nm):
                    ps = psF.tile([96, Q336], F32, name="psf", tag="psf")
                    nc.tensor.matmul(ps[:], w_sb[:, 0, cols], qT[0][:],
                                     start=True, stop=False)
                    nc.tensor.matmul(ps[:], w_sb[:, 1, cols], qT[1][:],
                                     start=False, stop=False)
                    nc.tensor.matmul(ps[:], bias_sb[:, cols], ones1[:],
                                     start=False, stop=True)
                    return ps

                logit_ps = head_mm(wattn_sb, battn_sb, slice(0, 96), "logit")
                expT = fs("expT")
                nc.scalar.activation(expT[:], logit_ps[:], EXP)
                s_ps = psF.tile([96, Q336], F32, name="psf", tag="psf")
                nc.tensor.matmul(s_ps[0:8, :], sel8[:], expT[:])
                rsum = fs("rsum")
                nc.vector.reciprocal(rsum[0:8, :], s_ps[0:8, :])
                rb_ps = psF.tile([96, Q336], F32, name="psf", tag="psf")
                nc.tensor.matmul(rb_ps[:], rep8[:], rsum[0:8, :])
                attnT = fs("attnT")
                nc.vector.tensor_tensor(attnT[:], expT[:], rb_ps[:], MUL)

                offx_ps = head_mm(woff_sb, boff_sb, slice(0, 192, 2), "offx")
                offx = fs("offx")
                nc.scalar.copy(offx[:], offx_ps[:])
                offy_ps = head_mm(woff_sb, boff_sb, slice(1, 192, 2), "offy")
                offy = fs("offy")
                nc.scalar.copy(offy[:], offy_ps[:])

                # --- x: pair-slot weights wA, wB; clipped base axb -------
                t1x = fs("t1x")
                nc.vector.tensor_tensor(t1x[:], offx[:], rwc[:], MUL)
                ixp = fs("ixp")
                nc.vector.scalar_tensor_tensor(
                    ixp[:], t1x[:], CAST_BIAS, rxw[:], ADD, ADD)
                fi = f16pool.tile([96, Q336], I16, name="fi", tag="fi")
                nc.vector.tensor_copy(fi[:], ixp[:])
                fxp = fs("fxp")
                nc.vector.tensor_copy(fxp[:], fi[:])
                a0x = fs("a0x")
                nc.vector.tensor_scalar(a0x[:], fxp[:], SHIFT, cvs(0), MAX, MIN)
                a1x = fs("a1x")
                nc.vector.tensor_scalar(a1x[:], fxp[:], SHIFT - 1.0, cvs(1),
                                        MAX, MIN)
                axb = fs("axb")
                nc.vector.tensor_scalar(axb[:], fxp[:], SHIFT, cvs(1), MAX, MIN)
                v0x = fs("v0x")
                nc.vector.tensor_tensor(v0x[:], a0x[:], fxp[:], EQ)
                v1x = fs("v1x")
                nc.vector.tensor_tensor(v1x[:], a1x[:], fxp[:], EQ)
                fx = fs("fx")
                nc.vector.scalar_tensor_tensor(
                    fx[:], ixp[:], SHIFT - CAST_BIAS - 0.5, fxp[:], ADD, SUB)
                omfx = fs("omfx")
                nc.vector.tensor_scalar(omfx[:], fx[:], -1.0, 1.0, MUL, ADD)
                wx0 = fs("wx0")
                nc.vector.tensor_tensor(wx0[:], omfx[:], v0x[:], MUL)
                wx1 = fs("wx1")
                nc.vector.tensor_tensor(wx1[:], fx[:], v1x[:], MUL)
                eqN = fs("eqN")
                nc.vector.tensor_tensor(eqN[:], fxp[:], axb[:], EQ)
                eqm1 = fs("eqm1")
                nc.vector.tensor_scalar(eqm1[:], fxp[:], SHIFT - 1.0, None, EQ)
                eqW1 = fs("eqW1")
                nc.vector.tensor_scalar(eqW1[:], fxp[:], cvs(0), None, EQ)
                u0 = fs("u0")
                nc.vector.tensor_tensor(u0[:], eqN[:], wx0[:], MUL)
                u1 = fs("u1")
                nc.vector.tensor_tensor(u1[:], eqm1[:], wx1[:], MUL)
                wA = fs("wA")
                nc.vector.tensor_tensor(wA[:], u0[:], u1[:], ADD)
                u2 = fs("u2")
                nc.vector.tensor_tensor(u2[:], eqN[:], wx1[:], MUL)
                u3 = fs("u3")
                nc.vector.tensor_tensor(u3[:], eqW1[:], wx0[:], MUL)
                wB = fs("wB")
                nc.vector.tensor_tensor(wB[:], u2[:], u3[:], ADD)

                # --- y: classic corner weights ---------------------------
                t1y = fs("t1y")
                nc.vector.tensor_tensor(t1y[:], offy[:], rhc[:], MUL)
                iyp = fs("iyp")
                nc.vector.scalar_tensor_tensor(
                    iyp[:], t1y[:], CAST_BIAS, ryw[:], ADD, ADD)
                fiy = f16pool.tile([96, Q336], I16, name="fiy", tag="fiy")
                nc.vector.tensor_copy(fiy[:], iyp[:])
                fyp = fs("fyp")
                nc.vector.tensor_copy(fyp[:], fiy[:])
                a0y = fs("a0y")
                nc.vector.tensor_scalar(a0y[:], fyp[:], SHIFT, cvs(0), MAX, MIN)
                a1y = fs("a1y")
                nc.vector.tensor_scalar(a1y[:], fyp[:], SHIFT - 1.0, cvs(1),
                                        MAX, MIN)
                v0y = fs("v0y")
                nc.vector.tensor_tensor(v0y[:], a0y[:], fyp[:], EQ)
                v1y = fs("v1y")
                nc.vector.tensor_tensor(v1y[:], a1y[:], fyp[:], EQ)
                fy = fs("fy")
                nc.vector.scalar_tensor_tensor(
                    fy[:], iyp[:], SHIFT - CAST_BIAS - 0.5, fyp[:], ADD, SUB)
                omfy = fs("omfy")
                nc.vector.tensor_scalar(omfy[:], fy[:], -1.0, 1.0, MUL, ADD)
                wy0 = fs("wy0")
                nc.vector.tensor_tensor(wy0[:], omfy[:], v0y[:], MUL)
                wy1 = fs("wy1")
                nc.vector.tensor_tensor(wy1[:], fy[:], v1y[:], MUL)
                wy0a = fs("wy0a")
                nc.vector.tensor_tensor(wy0a[:], wy0[:], attnT[:], MUL)
                wy1a = fs("wy1a")
                nc.vector.tensor_tensor(wy1a[:], wy1[:], attnT[:], MUL)
                # y pair-slot weights (L12 patch rows)
                ayb = fs("ayb")
                nc.vector.tensor_scalar(ayb[:], fyp[:], SHIFT, cvs(1), MAX, MIN)
                eqNy = fs("eqNy")
                nc.vector.tensor_tensor(eqNy[:], fyp[:], ayb[:], EQ)
                eqm1y = fs("eqm1y")
                nc.vector.tensor_scalar(eqm1y[:], fyp[:], SHIFT - 1.0, None, EQ)
                eqW1y = fs("eqW1y")
                nc.vector.tensor_scalar(eqW1y[:], fyp[:], cvs(0), None, EQ)
                uy0 = fs("uy0")
                nc.vector.tensor_tensor(uy0[:], eqNy[:], wy0[:], MUL)
                uy1 = fs("uy1")
                nc.vector.tensor_tensor(uy1[:], eqm1y[:], wy1[:], MUL)
                wyA = fs("wyA")
                nc.vector.tensor_tensor(wyA[:], uy0[:], uy1[:], ADD)
                uy2 = fs("uy2")
                nc.vector.tensor_tensor(uy2[:], eqNy[:], wy1[:], MUL)
                uy3 = fs("uy3")
                nc.vector.tensor_tensor(uy3[:], eqW1y[:], wy0[:], MUL)
                wyB = fs("wyB")
                nc.vector.tensor_tensor(wyB[:], uy2[:], uy3[:], ADD)
                wyAa = fs("wyAa")
                nc.vector.tensor_tensor(wyAa[:], wyA[:], attnT[:], MUL)
                wyBa = fs("wyBa")
                nc.vector.tensor_tensor(wyBa[:], wyB[:], attnT[:], MUL)

                # --- w8 [96, 2y, KG k, 16 a, 2 s] f32 --------------------
                w8 = wpool.tile([96, KG, 2, 16, 2], BF16, name="w8", tag="w8")
                qv = lambda t: t[:, 0:Q304].rearrange("c (k a) -> c k a", k=KG)
                for (y, wya) in ((0, wy0a), (1, wy1a)):
                    for (s, wab) in ((0, wA), (1, wB)):
                        pr = fs(f"w8p{y}{s}")
                        nc.vector.tensor_tensor(pr[:], wya[:], wab[:], MUL)
                        nc.vector.tensor_copy(w8[:, :, y, :, s], qv(pr))
                w8l = wpool.tile([96, KG, 16, 4], BF16, name="w8l", tag="w8l")
                for c, (wya, wab) in enumerate(
                        ((wyAa, wA), (wyAa, wB), (wyBa, wA), (wyBa, wB))):
                    pr = fs(f"w8l{c}")
                    nc.vector.tensor_tensor(pr[:], wya[:], wab[:], MUL)
                    nc.vector.tensor_copy(w8l[:, :, :, c], qv(pr))

                # --- pair idx per y corner -------------------------------
                yw0 = fs("yw0")
                nc.vector.tensor_scalar(yw0[:], a0y[:], cvs(2), None, MUL)
                yw1 = fs("yw1")
                nc.vector.tensor_scalar(yw1[:], a1y[:], cvs(2), None, MUL)
                lin16q = ipool.tile([QP, NSLOT, 8, 16], I16, name="lin16q",
                                    tag="lin16q")
                ybw = fs("ybw")
                nc.vector.tensor_scalar(ybw[:], ayb[:], cvs(2), None, MUL)
                idxP = fs("idxP")
                nc.vector.scalar_tensor_tensor(idxP[:], axb[:], cvs(4),
                                               ybw[:], ADD, ADD)
                for y, (yw, cl) in enumerate(((yw0, 3), (yw1, 5))):
                    lin = fs("lin")
                    nc.vector.scalar_tensor_tensor(lin[:], axb[:], cvs(cl),
                                                   yw[:], ADD, ADD)
                    mh = fs("mh")
                    nc.vector.tensor_scalar(mh[:], lin[:], 0.5, -0.25, MUL, ADD)
                    mi = f16pool.tile([96, Q336], I16, name="mi", tag="mi")
                    nc.vector.tensor_copy(mi[:], mh[:])
                    mf = fs("mf")
                    nc.vector.tensor_copy(mf[:], mi[:])
                    par = fs("par")
                    nc.vector.scalar_tensor_tensor(par[:], mf[:], -2.0, lin[:],
                                                   MUL, ADD)
                    idxf = fs("idxf")
                    nc.vector.scalar_tensor_tensor(idxf[:], par[:], float(NPAIR),
                                                   mf[:], MUL, ADD)
                    for slot in range(NSLOT):
                        pt = psS.tile([128, 128], F32, name="pss", tag="pss")
                        nc.tensor.transpose(
                            pt[0:QP, 0:96],
                            idxf[:, slot * QP:(slot + 1) * QP],
                            ident[0:96, 0:96])
                        nc.scalar.copy(
                            lin16q[:, slot, :, y:8:2],
                            pt[0:QP, 0:96].rearrange(
                                "r (h p) -> r h p", h=8)[:, :, 0:4])
                for slot in range(NSLOT):
                    pt = psS.tile([128, 128], F32, name="pss", tag="pss")
                    nc.tensor.transpose(
                        pt[0:QP, 0:96],
                        idxP[:, slot * QP:(slot + 1) * QP],
                        ident[0:96, 0:96])
                    nc.scalar.copy(
                        lin16q[:, slot, :, 8:16],
                        pt[0:QP, 0:96].rearrange(
                            "r (h p) -> r h p", h=8)[:, :, 4:12])

                # ==== 4. idx wrap via DRAM ===============================
                if "wrap" not in stages:
                    continue
                nc.sync.dma_start(
                    linq_d[b], lin16q[:].rearrange("r s h u -> r (s h u)"))
                # idxt[grp][g]: [128=(hl,dh,a), col = k*8 + u] i16
                idxt = [[], []]
                lsrc = linq_d[b].rearrange(
                    "(k a) (sl h u) -> a k sl h u", a=16, sl=NSLOT, h=8, u=16)
                for g in range(2):
                    for grp in range(2):
                        it = ipool.tile([128, NCOL], I16, name=f"idxt{grp}{g}",
                                        tag=f"idxt{grp}{g}")
                        dst = it[:].rearrange(
                            "(hl dh a) (k u) -> hl dh a k u", hl=4, dh=2, k=KG)
                        for hl in range(4):
                            for dh in range(2):
                                for sl in range(NSLOT):
                                    nk = 7 if sl < 2 else 5
                                    nc.sync.dma_start(
                                        dst[hl, dh, :, 7 * sl:7 * sl + nk, :],
                                        lsrc[:, 0:nk, sl, 4 * g + hl,
                                             8 * grp:8 * grp + 8])
                        idxt[grp].append(it)

                # ==== 5. gather ==========================================
                if "gather" not in stages:
                    continue
                Gs = {}
                for g in range(2):
                    for ich, (k0, k1) in enumerate(KCH):
                        nidx = (k1 - k0) * 8 * 16
                        G0 = gpool.tile([128, nidx, 2], BF16, name="G0",
                                        tag="G0")
                        nc.gpsimd.ap_gather(
                            G0[:], Th[g][:, 0:2 * S_PAD],
                            idxt[0][g][:, 8 * k0:8 * k1],
                            channels=128, num_elems=S_PAD, d=2, num_idxs=nidx)
                        G1 = gpool.tile([128, nidx, 4], BF16, name="G1",
                                        tag="G1", bufs=2)
                        nc.gpsimd.ap_gather(
                            G1[:], Th[g][:, PB:PB + 4 * NPATCH],
                            idxt[1][g][:, 8 * k0:8 * k1],
                            channels=128, num_elems=NPATCH, d=4, num_idxs=nidx)
                        Gs[(g, ich)] = (G0, G1)

                # ==== 6. weighted reduce =================================
                if "reduce" not in stages:
                    continue
                oh = [opool.tile([128, Q304], F32, name=f"oh{g}", tag=f"oh{g}")
                      for g in range(2)]
                for g in range(2):
                    acc0 = apool.tile([128, KG, 2, 16, 2], BF16, name="acc0",
                                      tag="acc0")
                    acc1 = apool.tile([128, KG, 16, 4], BF16, name="acc1",
                                      tag="acc1")
                    for ich, (k0, k1) in enumerate(KCH):
                        nk = k1 - k0
                        G0, G1 = Gs[(g, ich)]
                        gv0 = G0[:].rearrange("c (k p y a) s -> c k p y a s",
                                              k=nk, p=4, y=2)
                        gv1 = G1[:].rearrange("c (k p a) s -> c k p a s",
                                              k=nk, p=8)
                        for p in range(NP_TOT):
                            wp = psW.tile([128, nk, 64], F32, name="wp",
                                          tag="wp")
                            if p < 4:
                                rhs = w8[:, k0:k1, :, :, :].rearrange(
                                    "c k y a s -> c k (y a s)")
                                gsl = gv0[:, :, p].rearrange(
                                    "c k y a s -> c k (y a s)")
                                av = acc0[:, k0:k1].rearrange(
                                    "c k y a s -> c k (y a s)")
                            else:
                                rhs = w8l[:, k0:k1, :, :].rearrange(
                                    "c k a s -> c k (a s)")
                                gsl = gv1[:, :, p - 4].rearrange(
                                    "c k a s -> c k (a s)")
                                av = acc1[:, k0:k1].rearrange(
                                    "c k a s -> c k (a s)")
                            nc.tensor.matmul(
                                wp[:],
                                selp[:, (g * 12 + p) * 128:
                                     (g * 12 + p + 1) * 128], rhs)
                            wpb = bpool.tile([128, nk, 64], BF16, name="wpb",
                                             tag="wpb")
                            nc.scalar.copy(wpb[:], wp[:])
                            if p in (0, 4):
                                nc.vector.tensor_tensor(av, gsl, wpb[:], MUL)
                            else:
                                M = mpool.tile([128, nk, 64], BF16, name="M",
                                               tag="M")
                                nc.vector.tensor_tensor(M[:], gsl, wpb[:], MUL)
                                nc.vector.tensor_tensor(av, av, M[:], ADD)
                    ys = ypool.tile([128, KG, 16, 2], BF16, name="ys", tag="ys")
                    nc.vector.tensor_tensor(ys[:], acc0[:, :, 0], acc0[:, :, 1],
                                            ADD)
                    nc.vector.tensor_tensor(
                        oh[g][:].rearrange("c (k a) -> c k a", k=KG),
                        ys[:, :, :, 0], ys[:, :, :, 1], ADD)
                    v1t = ypool.tile([128, KG, 16, 2], BF16, name="v1t",
                                     tag="v1t")
                    nc.vector.tensor_tensor(v1t[:], acc1[:, :, :, 0:2],
                                            acc1[:, :, :, 2:4], ADD)
                    t2 = ypool.tile([128, KG, 16], BF16, name="t2", tag="t2")
                    nc.vector.tensor_tensor(t2[:], v1t[:, :, :, 0],
                                            v1t[:, :, :, 1], ADD)
                    ohv = oh[g][:].rearrange("c (k a) -> c k a", k=KG)
                    nc.vector.tensor_tensor(ohv, ohv, t2[:], ADD)

                # ==== 7. output ==========================================
                osb = opool.tile([QP, NSLOT, DM], F32, name="osb", tag="osb",
                                 bufs=1)
                for g in range(2):
                    for slot in range(NSLOT):
                        qn = QP if slot < 2 else Q304 - 2 * QP
                        pt = psS.tile([128, 128], F32, name="pss", tag="pss")
                        nc.tensor.transpose(
                            pt[0:qn, :],
                            oh[g][:, slot * QP:slot * QP + qn], ident[:])
                        nc.scalar.copy(
                            osb[0:qn, slot, g * 128:(g + 1) * 128], pt[0:qn, :])

                nc.sync.dma_start(
                    out_d[b][0:224].rearrange("(s r) c -> r s c", r=QP),
                    osb[:, 0:2, :])
                nc.sync.dma_start(out_d[b][224:300, :], osb[0:76, 2, :])
    return nc


_CACHE = {}


def _get_nc(repeat=None, stages=ALL_STAGES):
    if repeat is None:
        repeat = REPEAT
    key = ("nc", repeat, tuple(stages))
    if key not in _CACHE:
        nc = bacc.Bacc("TRN2", target_bir_lowering=False, debug=False,
                       enable_asserts=False)
        emit(nc, repeat=repeat, stages=stages)
        nc.compile()
        _CACHE[key] = nc
    return _CACHE[key]


def _in_maps(query, reference_points, input_flatten, W_off, b_off, W_attn,
             b_attn):
    query = np.asarray(query, np.float32)
    refp = np.asarray(reference_points, np.float32).reshape(B, LQ, 4)
    value = np.asarray(input_flatten, np.float32)
    consts = make_consts()
    in_maps = []
    for c in range(N_CORES):
        sl = slice(c * BPC, (c + 1) * BPC)
        in_maps.append({
            "value": value[sl], "query": query[sl], "refp": refp[sl],
            "woff": np.asarray(W_off, np.float32),
            "wattn": np.asarray(W_attn, np.float32),
            "boff": np.asarray(b_off, np.float32).reshape(1, 192),
            "battn": np.asarray(b_attn, np.float32).reshape(1, 96),
            **{k: v for k, v in consts.items()},
        })
    return in_maps


def kernel(query, reference_points, input_flatten, W_off, b_off, W_attn,
           b_attn):
    in_maps = _in_maps(query, reference_points, input_flatten, W_off, b_off,
                       W_attn, b_attn)
    res = run_bass_kernel_spmd(_get_nc(), in_maps, core_ids=list(range(N_CORES)))
    out = np.concatenate([r["out"] for r in res.results], axis=0)
    return out.astype(np.float32)


if __name__ == "__main__":
    import sys
    sys.path.insert(0, "/root/problem")
    import reference
    inputs = {k: np.asarray(v) for k, v in reference.setup_inputs().items()}
    got = kernel(**inputs)
    exp = np.asarray(reference.reference(**inputs))
    err = np.abs(got - exp).max() / np.abs(exp).max()
    print("Relative error:", err)



# revision 2
# speedup vs baseline: 2.2337x; 2.2337x over previous
"""DFine MultiScale Deformable Attention — Trainium2 Bass kernel, v3.

Data-parallel over batch (4 per core x 8 cores). Host packs `value` into a
bf16 patch table P[b, hp, h2, par, m, (32d, 2ys, 2xs)]: entry (par, m) holds
the 2x2 bilinear stencil starting at pixel s = 2m+par for one head. On
device, per batch:
  1. Frontend in [96=(h,p), 384=(3qb x 128q)] layout: offsets/attn via PE
     matmuls, softmax, x/y pair-slot weights (wA/wB, wyA/wyB), patch index
     idx = (h2*2+par)*4200 + m via the floor-parity trick.
  2. Per qb: PE transposes move weights + idx to query-partitions; idx is
     reordered to the SWDGE 16-partition wrap via 8 [96,16] transposes into
     PSUM + ACT reorder copies + log2 partition replication.
  3. 12 dma_gather calls (1024 descs x 256B, 4 SWDGE queues) fetch
     G[qp, (hp,pt,h2,p4), (d,ys,xs)] bf16 straight from HBM.
  4. One DVE 4x multiply against the d-broadcast weight AP + packed tree
     reduction over (ys, xs, p) -> out[qp, 256]; direct output DMA.
"""

import numpy as np
import ml_dtypes

import concourse.bass as bass
import concourse.tile as tile
from concourse import bacc, mybir, library_config
from concourse.bass_utils import run_bass_kernel_spmd

F32 = mybir.dt.float32
BF16 = mybir.dt.bfloat16
I16 = mybir.dt.int16
I32 = mybir.dt.int32

B, LQ, DM, NH, HD = 32, 300, 256, 8, 32
NP_TOT = 12
LVL_W = [80, 40, 20]
LVL_BASE = [0, 6400, 8000]
LVL_N = [6400, 1600, 400]
S = 8400
NPAIR = 4200
N_CORES = 8
BPC = B // N_CORES
SHIFT = 64.0
CAST_BIAS = SHIFT - 1.0
REPEAT = 1

Q384 = 384
NQB = 3

ALL_STAGES = ("front", "gather", "reduce")


def _wl(p):
    return float(LVL_W[p // 4])


def _base(p):
    return float(LVL_BASE[p // 4])


def make_consts():
    c = {}
    c["ident"] = np.eye(128, dtype=np.float32)
    sel8 = np.zeros((96, 8), np.float32)
    rep8 = np.zeros((8, 96), np.float32)
    for h in range(8):
        for p in range(NP_TOT):
            sel8[h * NP_TOT + p, h] = 1.0
            rep8[h, h * NP_TOT + p] = 1.0
    c["sel8"] = sel8
    c["rep8"] = rep8
    selr = np.zeros((4, 4 * 96), np.float32)
    for hp in range(96):
        p = hp % NP_TOT
        w = _wl(p)
        selr[0, 0 * 96 + hp] = w
        selr[1, 1 * 96 + hp] = w
        selr[2, 2 * 96 + hp] = 0.125 * w
        selr[3, 3 * 96 + hp] = 0.125 * w
    c["selr"] = selr
    c["ones1"] = np.ones((1, Q384), np.float32)
    cv = np.zeros((96, 8), np.float32)
    for hp in range(96):
        h, p = hp // NP_TOT, hp % NP_TOT
        w, base = _wl(p), _base(p)
        cv[hp, 0] = w - 1.0 + SHIFT                   # XMAX (x0/y0 max)
        cv[hp, 1] = w - 2.0 + SHIFT                   # XM2 (pair-base max)
        cv[hp, 2] = w                                 # W
        # patch-linear const: base - SHIFT*W - SHIFT + (h%2)*2*S
        cv[hp, 3] = base - SHIFT * w - SHIFT + float(h % 2) * 2.0 * S
    c["cv"] = cv
    return c


def make_patches(value):
    """value [BPC, S, 256] f32 -> P [BPC, 4, 2, 2, NPAIR, 128] bf16.

    P[b, hppair, h2, par, m, d*4 + ys*2 + xs] =
        value[b, (2m+par) + ys*W_l + xs, (2*hp+h2)*32 + d]
    (level l owning pixel 2m+par; reads past the level end are only indexed
    with clamped coordinates, so garbage there is never used).
    """
    bpc = value.shape[0]
    vb = value.astype(ml_dtypes.bfloat16)
    vpad = np.zeros((bpc, S + 96, 256), ml_dtypes.bfloat16)
    vpad[:, :S] = vb
    out = np.zeros((bpc, 4, 2, 2, NPAIR, 128), ml_dtypes.bfloat16)
    for l in range(3):
        w, base, npix = LVL_W[l], LVL_BASE[l], LVL_N[l]
        st = np.stack(
            [np.stack([vpad[:, base + ys * w + xs: base + ys * w + xs + npix]
                       for xs in range(2)], axis=2)
             for ys in range(2)], axis=2)
        for par in range(2):
            sel = st[:, par::2]
            cnt = sel.shape[1]
            m0 = (base + par) // 2
            selr_ = sel.reshape(bpc, cnt, 2, 2, 8, 32)
            pv = selr_.transpose(0, 4, 1, 5, 2, 3)  # [b, h, cnt, d, ys, xs]
            pv = pv.reshape(bpc, 4, 2, cnt, 32, 2, 2)  # [b, hp, h2, cnt,...]
            out[:, :, :, par, m0:m0 + cnt] = pv.reshape(
                bpc, 4, 2, cnt, 128)
    return out


def emit(nc, repeat=None, stages=ALL_STAGES):
    if repeat is None:
        repeat = REPEAT
    stages = set(stages)

    patches = nc.dram_tensor("patches", [BPC, 4, 2, 2, NPAIR, 128], BF16,
                             kind="ExternalInput").ap()
    query = nc.dram_tensor("query", [BPC, LQ, DM], F32,
                           kind="ExternalInput").ap()
    refp = nc.dram_tensor("refp", [BPC, LQ, 4], F32,
                          kind="ExternalInput").ap()
    woff = nc.dram_tensor("woff", [DM, 192], F32, kind="ExternalInput").ap()
    wattn = nc.dram_tensor("wattn", [DM, 96], F32, kind="ExternalInput").ap()
    boff = nc.dram_tensor("boff", [1, 192], F32, kind="ExternalInput").ap()
    battn = nc.dram_tensor("battn", [1, 96], F32, kind="ExternalInput").ap()
    ident_d = nc.dram_tensor("ident", [128, 128], F32,
                             kind="ExternalInput").ap()
    sel8_d = nc.dram_tensor("sel8", [96, 8], F32, kind="ExternalInput").ap()
    rep8_d = nc.dram_tensor("rep8", [8, 96], F32, kind="ExternalInput").ap()
    selr_d = nc.dram_tensor("selr", [4, 4 * 96], F32,
                            kind="ExternalInput").ap()
    ones1_d = nc.dram_tensor("ones1", [1, Q384], F32,
                             kind="ExternalInput").ap()
    cv_d = nc.dram_tensor("cv", [96, 8], F32, kind="ExternalInput").ap()
    out_d = nc.dram_tensor("out", [BPC, LQ, DM], F32,
                           kind="ExternalOutput").ap()

    MUL, ADD, SUB, MAX, MIN, EQ = (
        mybir.AluOpType.mult, mybir.AluOpType.add, mybir.AluOpType.subtract,
        mybir.AluOpType.max, mybir.AluOpType.min, mybir.AluOpType.is_equal)
    EXP = mybir.ActivationFunctionType.Exp

    with tile.TileContext(nc) as tc:
        import contextlib
        ctx = contextlib.ExitStack()
        with ctx:
            cpool = ctx.enter_context(tc.tile_pool(name="consts", bufs=1))
            fpool = ctx.enter_context(tc.tile_pool(name="front", bufs=14))
            ipool = ctx.enter_context(tc.tile_pool(name="idx", bufs=2))
            i32pool = ctx.enter_context(tc.tile_pool(name="i32", bufs=1))
            wpool = ctx.enter_context(tc.tile_pool(name="wb", bufs=2))
            gpool = ctx.enter_context(tc.tile_pool(name="gath", bufs=2))
            mpool = ctx.enter_context(tc.tile_pool(name="mm", bufs=1))
            rpool = ctx.enter_context(tc.tile_pool(name="red", bufs=2))
            opool = ctx.enter_context(tc.tile_pool(name="outsb", bufs=2))
            qpool = ctx.enter_context(tc.tile_pool(name="qt", bufs=1))
            psS = ctx.enter_context(
                tc.tile_pool(name="psS", bufs=2, space="PSUM"))
            psF = ctx.enter_context(
                tc.tile_pool(name="psF", bufs=2, space="PSUM"))
            psI = ctx.enter_context(
                tc.tile_pool(name="psI", bufs=2, space="PSUM"))

            nc.gpsimd.load_library(library_config.attnmlp)

            def ld(dst, src):
                nc.sync.dma_start(dst, src)

            ident = cpool.tile([128, 128], F32, name="ident")
            ld(ident[:], ident_d)
            sel8 = cpool.tile([96, 8], F32, name="sel8")
            ld(sel8[:], sel8_d)
            rep8 = cpool.tile([8, 96], F32, name="rep8")
            ld(rep8[:], rep8_d)
            selr = cpool.tile([4, 4 * 96], F32, name="selr")
            ld(selr[:], selr_d)
            ones1 = cpool.tile([1, Q384], F32, name="ones1")
            ld(ones1[:], ones1_d)
            cv = cpool.tile([96, 8], F32, name="cv")
            ld(cv[:], cv_d)
            woff_sb = cpool.tile([128, 2, 192], F32, name="woff_sb")
            ld(woff_sb[:], woff.rearrange("(kt p) m -> p kt m", p=128))
            wattn_sb = cpool.tile([128, 2, 96], F32, name="wattn_sb")
            ld(wattn_sb[:], wattn.rearrange("(kt p) m -> p kt m", p=128))
            boff_sb = cpool.tile([1, 192], F32, name="boff_sb")
            ld(boff_sb[:], boff)
            battn_sb = cpool.tile([1, 96], F32, name="battn_sb")
            ld(battn_sb[:], battn)

            def cvs(k):
                return cv[:, k:k + 1]

            def fs(nm):
                return fpool.tile([96, Q384], F32, name=nm, tag="fs")

            qi = 0  # SWDGE queue rotation

            for b4 in range(BPC * repeat):
                b = b4 % BPC
                if "front" not in stages:
                    continue
                # ==== 1. frontend ========================================
                qsb = qpool.tile([128, NQB, DM], F32, name="qsb", tag="qsb")
                nc.vector.memset(qsb[:, 2, :], 0.0)
                nc.sync.dma_start(
                    qsb[:, 0:2, :],
                    query[b][0:256].rearrange("(s r) c -> r s c", r=128))
                nc.sync.dma_start(qsb[0:44, 2, :], query[b][256:300, :])
                refsb = qpool.tile([128, NQB, 4], F32, name="refsb",
                                   tag="refsb")
                nc.vector.memset(refsb[:, 2, :], 0.0)
                nc.sync.dma_start(
                    refsb[:, 0:2, :],
                    refp[b][0:256].rearrange("(s r) c -> r s c", r=128))
                nc.sync.dma_start(refsb[0:44, 2, :], refp[b][256:300, :])

                qT = [qpool.tile([128, Q384], F32, name=f"qT{kt}",
                                 tag=f"qT{kt}") for kt in range(2)]
                for slot in range(NQB):
                    for kt in range(2):
                        pt = psS.tile([128, 128], F32, name="pss", tag="pss")
                        nc.tensor.transpose(
                            pt[:], qsb[:, slot, kt * 128:(kt + 1) * 128],
                            ident[:])
                        nc.scalar.copy(qT[kt][:, slot * 128:(slot + 1) * 128],
                                       pt[:])
                refT = fs("refT")
                for slot in range(NQB):
                    pt = psS.tile([128, 128], F32, name="pss", tag="pss")
                    nc.tensor.transpose(pt[0:4, :], refsb[:, slot, :],
                                        ident[:])
                    nc.scalar.copy(refT[0:4, slot * 128:(slot + 1) * 128],
                                   pt[0:4, :])

                refb = []
                for m in range(4):
                    ps = psF.tile([96, Q384], F32, name="psf", tag="psf")
                    nc.tensor.matmul(ps[:], selr[:, m * 96:(m + 1) * 96],
                                     refT[0:4, :])
                    sb = fs(f"refb{m}")
                    nc.scalar.copy(sb[:], ps[:])
                    refb.append(sb)
                rxw, ryw, rwc, rhc = refb

                def head_mm(w_sb, bias_sb, cols):
                    ps = psF.tile([96, Q384], F32, name="psf", tag="psf")
                    nc.tensor.matmul(ps[:], w_sb[:, 0, cols], qT[0][:],
                                     start=True, stop=False)
                    nc.tensor.matmul(ps[:], w_sb[:, 1, cols], qT[1][:],
                                     start=False, stop=False)
                    nc.tensor.matmul(ps[:], bias_sb[:, cols], ones1[:],
                                     start=False, stop=True)
                    return ps

                logit_ps = head_mm(wattn_sb, battn_sb, slice(0, 96))
                expT = fs("expT")
                nc.scalar.activation(expT[:], logit_ps[:], EXP)
                s_ps = psF.tile([96, Q384], F32, name="psf", tag="psf")
                nc.tensor.matmul(s_ps[0:8, :], sel8[:], expT[:])
                rsum = fs("rsum")
                nc.vector.reciprocal(rsum[0:8, :], s_ps[0:8, :])
                rb_ps = psF.tile([96, Q384], F32, name="psf", tag="psf")
                nc.tensor.matmul(rb_ps[:], rep8[:], rsum[0:8, :])
                attnT = fs("attnT")
                nc.vector.tensor_tensor(attnT[:], expT[:], rb_ps[:], MUL)

                offx_ps = head_mm(woff_sb, boff_sb, slice(0, 192, 2))
                offx = fs("offx")
                nc.scalar.copy(offx[:], offx_ps[:])
                offy_ps = head_mm(woff_sb, boff_sb, slice(1, 192, 2))
                offy = fs("offy")
                nc.scalar.copy(offy[:], offy_ps[:])

                # --- x: pair-slot weights wA, wB; clipped base axb -------
                t1x = fs("t1x")
                nc.vector.tensor_tensor(t1x[:], offx[:], rwc[:], MUL)
                ixp = fs("ixp")
                nc.vector.scalar_tensor_tensor(
                    ixp[:], t1x[:], CAST_BIAS, rxw[:], ADD, ADD)
                fi = i32pool.tile([96, Q384], I32, name="fi", tag="fi")
                nc.vector.tensor_copy(fi[:], ixp[:])
                fxp = fs("fxp")
                nc.vector.tensor_copy(fxp[:], fi[:])
                a0x = fs("a0x")
                nc.vector.tensor_scalar(a0x[:], fxp[:], SHIFT, cvs(0),
                                        MAX, MIN)
                a1x = fs("a1x")
                nc.vector.tensor_scalar(a1x[:], fxp[:], SHIFT - 1.0, cvs(1),
                                        MAX, MIN)
                axb = fs("axb")
                nc.vector.tensor_scalar(axb[:], fxp[:], SHIFT, cvs(1),
                                        MAX, MIN)
                v0x = fs("v0x")
                nc.vector.tensor_tensor(v0x[:], a0x[:], fxp[:], EQ)
                v1x = fs("v1x")
                nc.vector.tensor_tensor(v1x[:], a1x[:], fxp[:], EQ)
                fx = fs("fx")
                nc.vector.scalar_tensor_tensor(
                    fx[:], ixp[:], SHIFT - CAST_BIAS - 0.5, fxp[:], ADD, SUB)
                omfx = fs("omfx")
                nc.vector.tensor_scalar(omfx[:], fx[:], -1.0, 1.0, MUL, ADD)
                wx0 = fs("wx0")
                nc.vector.tensor_tensor(wx0[:], omfx[:], v0x[:], MUL)
                wx1 = fs("wx1")
                nc.vector.tensor_tensor(wx1[:], fx[:], v1x[:], MUL)
                eqN = fs("eqN")
                nc.vector.tensor_tensor(eqN[:], fxp[:], axb[:], EQ)
                eqm1 = fs("eqm1")
                nc.vector.tensor_scalar(eqm1[:], fxp[:], SHIFT - 1.0, None,
                                        EQ)
                eqW1 = fs("eqW1")
                nc.vector.tensor_scalar(eqW1[:], fxp[:], cvs(0), None, EQ)
                u0 = fs("u0")
                nc.vector.tensor_tensor(u0[:], eqN[:], wx0[:], MUL)
                u1 = fs("u1")
                nc.vector.tensor_tensor(u1[:], eqm1[:], wx1[:], MUL)
                wA = fs("wA")
                nc.vector.tensor_tensor(wA[:], u0[:], u1[:], ADD)
                u2 = fs("u2")
                nc.vector.tensor_tensor(u2[:], eqN[:], wx1[:], MUL)
                u3 = fs("u3")
                nc.vector.tensor_tensor(u3[:], eqW1[:], wx0[:], MUL)
                wB = fs("wB")
                nc.vector.tensor_tensor(wB[:], u2[:], u3[:], ADD)

                # --- y: pair-slot weights wyA, wyB; base ayb -------------
                t1y = fs("t1y")
                nc.vector.tensor_tensor(t1y[:], offy[:], rhc[:], MUL)
                iyp = fs("iyp")
                nc.vector.scalar_tensor_tensor(
                    iyp[:], t1y[:], CAST_BIAS, ryw[:], ADD, ADD)
                fiy = i32pool.tile([96, Q384], I32, name="fiy", tag="fiy")
                nc.vector.tensor_copy(fiy[:], iyp[:])
                fyp = fs("fyp")
                nc.vector.tensor_copy(fyp[:], fiy[:])
                a0y = fs("a0y")
                nc.vector.tensor_scalar(a0y[:], fyp[:], SHIFT, cvs(0),
                                        MAX, MIN)
                a1y = fs("a1y")
                nc.vector.tensor_scalar(a1y[:], fyp[:], SHIFT - 1.0, cvs(1),
                                        MAX, MIN)
                ayb = fs("ayb")
                nc.vector.tensor_scalar(ayb[:], fyp[:], SHIFT, cvs(1),
                                        MAX, MIN)
                v0y = fs("v0y")
                nc.vector.tensor_tensor(v0y[:], a0y[:], fyp[:], EQ)
                v1y = fs("v1y")
                nc.vector.tensor_tensor(v1y[:], a1y[:], fyp[:], EQ)
                fy = fs("fy")
                nc.vector.scalar_tensor_tensor(
                    fy[:], iyp[:], SHIFT - CAST_BIAS - 0.5, fyp[:], ADD, SUB)
                omfy = fs("omfy")
                nc.vector.tensor_scalar(omfy[:], fy[:], -1.0, 1.0, MUL, ADD)
                wy0 = fs("wy0")
                nc.vector.tensor_tensor(wy0[:], omfy[:], v0y[:], MUL)
                wy1 = fs("wy1")
                nc.vector.tensor_tensor(wy1[:], fy[:], v1y[:], MUL)
                eqNy = fs("eqNy")
                nc.vector.tensor_tensor(eqNy[:], fyp[:], ayb[:], EQ)
                eqm1y = fs("eqm1y")
                nc.vector.tensor_scalar(eqm1y[:], fyp[:], SHIFT - 1.0, None,
                                        EQ)
                eqW1y = fs("eqW1y")
                nc.vector.tensor_scalar(eqW1y[:], fyp[:], cvs(0), None, EQ)
                uy0 = fs("uy0")
                nc.vector.tensor_tensor(uy0[:], eqNy[:], wy0[:], MUL)
                uy1 = fs("uy1")
                nc.vector.tensor_tensor(uy1[:], eqm1y[:], wy1[:], MUL)
                wyA = fs("wyA")
                nc.vector.tensor_tensor(wyA[:], uy0[:], uy1[:], ADD)
                uy2 = fs("uy2")
                nc.vector.tensor_tensor(uy2[:], eqNy[:], wy1[:], MUL)
                uy3 = fs("uy3")
                nc.vector.tensor_tensor(uy3[:], eqW1y[:], wy0[:], MUL)
                wyB = fs("wyB")
                nc.vector.tensor_tensor(wyB[:], uy2[:], uy3[:], ADD)
                wyAa = fs("wyAa")
                nc.vector.tensor_tensor(wyAa[:], wyA[:], attnT[:], MUL)
                wyBa = fs("wyBa")
                nc.vector.tensor_tensor(wyBa[:], wyB[:], attnT[:], MUL)

                # --- weight products per patch corner (ys, xs) -----------
                w4 = []
                for (wya, wxs) in ((wyAa, wA), (wyAa, wB),
                                   (wyBa, wA), (wyBa, wB)):
                    pr = fs("w4")
                    nc.vector.tensor_tensor(pr[:], wya[:], wxs[:], MUL)
                    w4.append(pr)

                # --- patch index: idx = par*NPAIR + floor(lin2/2) --------
                ybw = fs("ybw")
                nc.vector.tensor_scalar(ybw[:], ayb[:], cvs(2), None, MUL)
                lin2 = fs("lin2")
                nc.vector.scalar_tensor_tensor(lin2[:], axb[:], cvs(3),
                                               ybw[:], ADD, ADD)
                mh = fs("mh")
                nc.vector.tensor_scalar(mh[:], lin2[:], 0.5, -0.25, MUL, ADD)
                mi = i32pool.tile([96, Q384], I32, name="mi", tag="mi")
                nc.vector.tensor_copy(mi[:], mh[:])
                mf = fs("mf")
                nc.vector.tensor_copy(mf[:], mi[:])
                par = fs("par")
                nc.vector.scalar_tensor_tensor(par[:], mf[:], -2.0, lin2[:],
                                               MUL, ADD)
                idxf = fs("idxf")
                nc.vector.scalar_tensor_tensor(idxf[:], par[:], float(NPAIR),
                                               mf[:], MUL, ADD)

                osb = opool.tile([128, NQB, DM], F32, name="osb", tag="osb")

                for qb in range(NQB):
                    cols = slice(qb * 128, (qb + 1) * 128)
                    # --- weights to q-partitions: Wb [128, 384] bf16 -----
                    # free layout (hp4, pt3, h2 2, p4 4, ys2, xs2)
                    Wb = wpool.tile([128, 384], BF16, name="Wb", tag="Wb")
                    for comp in range(4):
                        ys, xs = comp // 2, comp % 2
                        ptw = psS.tile([128, 128], F32, name="pss",
                                       tag="pss")
                        nc.tensor.transpose(ptw[:, 0:96], w4[comp][:, cols],
                                            ident[0:96, 0:96])
                        for h2 in range(2):
                            nc.scalar.copy(
                                Wb[:].rearrange(
                                    "r (hp pt h2 p4 ys xs) -> "
                                    "r hp pt h2 p4 ys xs",
                                    hp=4, pt=3, h2=2, p4=4,
                                    ys=2)[:, :, :, h2, :, ys, xs],
                                ptw[:, 0:96].rearrange(
                                    "r (h pt p4) -> r h pt p4",
                                    h=8, pt=3)[:, h2::2, :, :])

                    if "gather" not in stages:
                        continue

                    # --- idx to 16-part wrap: idxq [128, 768] i16 --------
                    psx = psI.tile([16, 8, 128], F32, name="psx", tag="psx")
                    for g8 in range(8):
                        nc.tensor.transpose(
                            psx[0:16, g8, 0:96],
                            idxf[:, qb * 128 + g8 * 16:
                                 qb * 128 + g8 * 16 + 16],
                            ident[0:96, 0:96])
                    idxq = ipool.tile([128, 768], I16, name="idxq",
                                      tag="idxq")
                    for hp in range(4):
                        for h2 in range(2):
                            h = hp * 2 + h2
                            nc.scalar.copy(
                                idxq[0:16].rearrange(
                                    "r (hp pt h2 p4 g8) -> "
                                    "r hp pt h2 p4 g8",
                                    hp=4, pt=3, h2=2,
                                    p4=4)[:, hp, :, h2, :, :],
                                psx[0:16, :, h * 12:h * 12 + 12].rearrange(
                                    "r g8 (pt p4) -> r pt p4 g8", pt=3))
                    # replicate idx to all 128 partitions (log doubling)
                    nc.sync.dma_start(idxq[16:32, :], idxq[0:16, :])
                    nc.sync.dma_start(idxq[32:64, :], idxq[0:32, :])
                    nc.sync.dma_start(idxq[64:128, :], idxq[0:64, :])

                    # --- 12 dma_gather calls -----------------------------
                    G = gpool.tile([128, 96, 128], BF16, name="G", tag="G")
                    for hp in range(4):
                        src = patches[b][hp].rearrange(
                            "h2 par m e -> (h2 par m) e")
                        for pt in range(3):
                            nc.gpsimd.dma_gather(
                                G[:, (hp * 3 + pt) * 8:
                                  (hp * 3 + pt) * 8 + 8, :],
                                src,
                                idxq[:, (hp * 3 + pt) * 64:
                                     (hp * 3 + pt) * 64 + 64],
                                num_idxs=1024, num_idxs_reg=1024,
                                elem_size=128, queue_num=qi % 4)
                            qi += 1
                    # G free layout: (hp, pt, h2, p4) x (d32, ys2, xs2)

                    if "reduce" not in stages:
                        continue

                    # --- MAC: M = G * broadcast(Wb) ----------------------
                    M = mpool.tile([128, 96, 32, 4], BF16, name="M", tag="M")
                    Gv = G[:].rearrange("r k (d s) -> r k d s", d=32)
                    Wv = Wb[:].rearrange("r (k s) -> r k s", s=4)
                    Wvb = Wv.unsqueeze(2).broadcast_to([128, 96, 32, 4])
                    nc.vector.tensor_tensor(M[:], Gv, Wvb, MUL)

                    # --- reduce over (ys, xs, p) -------------------------
                    # r1: fold ys (last-dim pairs [0:2]+[2:4])
                    R1 = rpool.tile([128, 96, 32, 2], BF16, name="R1",
                                    tag="R1")
                    nc.vector.tensor_tensor(R1[:], M[:, :, :, 0:2],
                                            M[:, :, :, 2:4], ADD)
                    # r1b: fold p4 pairs {0,1}+{2,3}
                    R1v = R1[:].rearrange(
                        "r (hpt p4) d s -> r hpt p4 (d s)", p4=4)
                    R1b = rpool.tile([128, 24, 2, 64], BF16, name="R1b",
                                     tag="R1b")
                    nc.vector.tensor_tensor(R1b[:], R1v[:, :, 0:2, :],
                                            R1v[:, :, 2:4, :], ADD)
                    # r1c: fold p4 remainder
                    R1c = rpool.tile([128, 24, 64], BF16, name="R1c",
                                     tag="R1c")
                    nc.vector.tensor_tensor(R1c[:], R1b[:, :, 0, :],
                                            R1b[:, :, 1, :], ADD)
                    # r2: fold xs (stride-2) -> f32
                    R2 = rpool.tile([128, 24, 32], F32, name="R2", tag="R2")
                    R1cv = R1c[:].rearrange("r k (d s) -> r k d s", s=2)
                    nc.vector.tensor_tensor(R2[:], R1cv[:, :, :, 0],
                                            R1cv[:, :, :, 1], ADD)
                    # fold pt (3): out osb[:, qb, :]
                    R2v = R2[:].rearrange("r (hp pt h2) d -> r hp pt (h2 d)",
                                          hp=4, pt=3)
                    T1 = rpool.tile([128, 4, 64], F32, name="T1", tag="T1")
                    nc.vector.tensor_tensor(T1[:], R2v[:, :, 0, :],
                                            R2v[:, :, 1, :], ADD)
                    nc.vector.tensor_tensor(
                        osb[:, qb, :].rearrange("r (hp e) -> r hp e", hp=4),
                        T1[:], R2v[:, :, 2, :], ADD)

                # ==== output =============================================
                if "reduce" in stages:
                    nc.sync.dma_start(
                        out_d[b][0:256].rearrange("(s r) c -> r s c", r=128),
                        osb[:, 0:2, :])
                    nc.sync.dma_start(out_d[b][256:300, :], osb[0:44, 2, :])
    return nc


_CACHE = {}


def _get_nc(repeat=None, stages=ALL_STAGES):
    if repeat is None:
        repeat = REPEAT
    key = ("nc", repeat, tuple(stages))
    if key not in _CACHE:
        nc = bacc.Bacc("TRN2", target_bir_lowering=False, debug=False,
                       enable_asserts=False, num_swdge_queues=4)
        emit(nc, repeat=repeat, stages=stages)
        nc.compile()
        _CACHE[key] = nc
    return _CACHE[key]


def _in_maps(query, reference_points, input_flatten, W_off, b_off, W_attn,
             b_attn):
    query = np.asarray(query, np.float32)
    refp = np.asarray(reference_points, np.float32).reshape(B, LQ, 4)
    value = np.asarray(input_flatten, np.float32)
    consts = make_consts()
    in_maps = []
    for c in range(N_CORES):
        sl = slice(c * BPC, (c + 1) * BPC)
        in_maps.append({
            "patches": make_patches(value[sl]),
            "query": query[sl], "refp": refp[sl],
            "woff": np.asarray(W_off, np.float32),
            "wattn": np.asarray(W_attn, np.float32),
            "boff": np.asarray(b_off, np.float32).reshape(1, 192),
            "battn": np.asarray(b_attn, np.float32).reshape(1, 96),
            **{k: v for k, v in consts.items()},
        })
    return in_maps


def kernel(query, reference_points, input_flatten, W_off, b_off, W_attn,
           b_attn):
    in_maps = _in_maps(query, reference_points, input_flatten, W_off, b_off,
                       W_attn, b_attn)
    res = run_bass_kernel_spmd(_get_nc(), in_maps,
                               core_ids=list(range(N_CORES)))
    out = np.concatenate([r["out"] for r in res.results], axis=0)
    return out.astype(np.float32)


if __name__ == "__main__":
    import sys
    sys.path.insert(0, "/root/problem")
    import reference
    inputs = {k: np.asarray(v) for k, v in reference.setup_inputs().items()}
    got = kernel(**inputs)
    exp = np.asarray(reference.reference(**inputs))
    err = np.abs(got - exp).max() / np.abs(exp).max()
    print("Relative error:", err)


# revision 5
# speedup vs baseline: 3.2987x; 1.4768x over previous
"""DFine MultiScale Deformable Attention — Trainium2 Bass kernel, v3.

Data-parallel over batch (4 per core x 8 cores). Host packs `value` into a
bf16 patch table P[b, hp, h2, par, m, (32d, 2ys, 2xs)]: entry (par, m) holds
the 2x2 bilinear stencil starting at pixel s = 2m+par for one head. On
device, per batch:
  1. Frontend in [96=(h,p), 384=(3qb x 128q)] layout: offsets/attn via PE
     matmuls, softmax, x/y pair-slot weights (wA/wB, wyA/wyB), patch index
     idx = (h2*2+par)*4200 + m via the floor-parity trick.
  2. Per qb: PE transposes move weights + idx to query-partitions; idx is
     reordered to the SWDGE 16-partition wrap via 8 [96,16] transposes into
     PSUM + ACT reorder copies + log2 partition replication.
  3. 12 dma_gather calls (1024 descs x 256B, 4 SWDGE queues) fetch
     G[qp, (hp,pt,h2,p4), (d,ys,xs)] bf16 straight from HBM.
  4. One DVE 4x multiply against the d-broadcast weight AP + packed tree
     reduction over (ys, xs, p) -> out[qp, 256]; direct output DMA.
"""

import numpy as np
import ml_dtypes

import concourse.bass as bass
import concourse.tile as tile
from concourse import bacc, mybir, library_config
from concourse.bass_utils import run_bass_kernel_spmd

F32 = mybir.dt.float32
BF16 = mybir.dt.bfloat16
I16 = mybir.dt.int16
I32 = mybir.dt.int32

B, LQ, DM, NH, HD = 32, 300, 256, 8, 32
NP_TOT = 12
LVL_W = [80, 40, 20]
LVL_BASE = [0, 6400, 8000]
LVL_N = [6400, 1600, 400]
S = 8400
NPAIR = 4200
N_CORES = 8
BPC = B // N_CORES
SHIFT = 64.0
CAST_BIAS = SHIFT - 1.0
REPEAT = 1

Q384 = 384
NQB = 3

ALL_STAGES = ("front", "gather", "reduce")


def _wl(p):
    return float(LVL_W[p // 4])


def _base(p):
    return float(LVL_BASE[p // 4])


def make_consts():
    c = {}
    c["ident"] = np.eye(128, dtype=np.float32)
    sel8 = np.zeros((96, 8), np.float32)
    rep8 = np.zeros((8, 96), np.float32)
    for h in range(8):
        for p in range(NP_TOT):
            sel8[h * NP_TOT + p, h] = 1.0
            rep8[h, h * NP_TOT + p] = 1.0
    c["sel8"] = sel8
    c["rep8"] = rep8
    selr = np.zeros((4, 4 * 96), np.float32)
    for hp in range(96):
        p = hp % NP_TOT
        w = _wl(p)
        selr[0, 0 * 96 + hp] = w
        selr[1, 1 * 96 + hp] = w
        selr[2, 2 * 96 + hp] = 0.125 * w
        selr[3, 3 * 96 + hp] = 0.125 * w
    c["selr"] = selr
    c["ones1"] = np.ones((1, Q384), np.float32)
    cv = np.zeros((96, 8), np.float32)
    for hp in range(96):
        h, p = hp // NP_TOT, hp % NP_TOT
        w, base = _wl(p), _base(p)
        cv[hp, 0] = w - 1.0 + SHIFT                   # XMAX (x0/y0 max)
        cv[hp, 1] = w - 2.0 + SHIFT                   # XM2 (pair-base max)
        cv[hp, 2] = w                                 # W
        # patch-linear const: base - SHIFT*W - SHIFT + (h%2)*2*S
        cv[hp, 3] = base - SHIFT * w - SHIFT + float(h % 2) * 2.0 * S
    c["cv"] = cv
    return c


def make_patches(value):
    """value [BPC, S, 256] f32 -> P [BPC, 4, 2, 2, NPAIR, 128] bf16.

    P[b, hppair, h2, par, m, d*4 + ys*2 + xs] =
        value[b, (2m+par) + ys*W_l + xs, (2*hp+h2)*32 + d]
    (level l owning pixel 2m+par; reads past the level end are only indexed
    with clamped coordinates, so garbage there is never used).
    """
    bpc = value.shape[0]
    vb = value.astype(ml_dtypes.bfloat16)
    vpad = np.zeros((bpc, S + 96, 256), ml_dtypes.bfloat16)
    vpad[:, :S] = vb
    out = np.zeros((bpc, 4, 2, 2, NPAIR, 128), ml_dtypes.bfloat16)
    for l in range(3):
        w, base, npix = LVL_W[l], LVL_BASE[l], LVL_N[l]
        st = np.stack(
            [np.stack([vpad[:, base + ys * w + xs: base + ys * w + xs + npix]
                       for xs in range(2)], axis=2)
             for ys in range(2)], axis=2)
        for par in range(2):
            sel = st[:, par::2]
            cnt = sel.shape[1]
            m0 = (base + par) // 2
            selr_ = sel.reshape(bpc, cnt, 2, 2, 8, 32)
            pv = selr_.transpose(0, 4, 1, 5, 2, 3)  # [b, h, cnt, d, ys, xs]
            pv = pv.reshape(bpc, 4, 2, cnt, 32, 2, 2)  # [b, hp, h2, cnt,...]
            out[:, :, :, par, m0:m0 + cnt] = pv.reshape(
                bpc, 4, 2, cnt, 128)
    return out


def emit(nc, repeat=None, stages=ALL_STAGES):
    if repeat is None:
        repeat = REPEAT
    stages = set(stages)

    patches = nc.dram_tensor("patches", [BPC, 4, 2, 2, NPAIR, 128], BF16,
                             kind="ExternalInput").ap()
    query = nc.dram_tensor("query", [BPC, LQ, DM], F32,
                           kind="ExternalInput").ap()
    refp = nc.dram_tensor("refp", [BPC, LQ, 4], F32,
                          kind="ExternalInput").ap()
    woff = nc.dram_tensor("woff", [DM, 192], F32, kind="ExternalInput").ap()
    wattn = nc.dram_tensor("wattn", [DM, 96], F32, kind="ExternalInput").ap()
    boff = nc.dram_tensor("boff", [1, 192], F32, kind="ExternalInput").ap()
    battn = nc.dram_tensor("battn", [1, 96], F32, kind="ExternalInput").ap()
    ident_d = nc.dram_tensor("ident", [128, 128], F32,
                             kind="ExternalInput").ap()
    sel8_d = nc.dram_tensor("sel8", [96, 8], F32, kind="ExternalInput").ap()
    rep8_d = nc.dram_tensor("rep8", [8, 96], F32, kind="ExternalInput").ap()
    selr_d = nc.dram_tensor("selr", [4, 4 * 96], F32,
                            kind="ExternalInput").ap()
    ones1_d = nc.dram_tensor("ones1", [1, Q384], F32,
                             kind="ExternalInput").ap()
    cv_d = nc.dram_tensor("cv", [96, 8], F32, kind="ExternalInput").ap()
    out_d = nc.dram_tensor("out", [BPC, LQ, DM], F32,
                           kind="ExternalOutput").ap()

    MUL, ADD, SUB, MAX, MIN, EQ = (
        mybir.AluOpType.mult, mybir.AluOpType.add, mybir.AluOpType.subtract,
        mybir.AluOpType.max, mybir.AluOpType.min, mybir.AluOpType.is_equal)
    EXP = mybir.ActivationFunctionType.Exp

    with tile.TileContext(nc) as tc:
        import contextlib
        ctx = contextlib.ExitStack()
        with ctx:
            cpool = ctx.enter_context(tc.tile_pool(name="consts", bufs=1))
            fpool = ctx.enter_context(tc.tile_pool(name="front", bufs=14))
            ipool = ctx.enter_context(tc.tile_pool(name="idx", bufs=2))
            i32pool = ctx.enter_context(tc.tile_pool(name="i32", bufs=1))
            wpool = ctx.enter_context(tc.tile_pool(name="wb", bufs=2))
            gpool = ctx.enter_context(tc.tile_pool(name="gath", bufs=2))
            mpool = ctx.enter_context(tc.tile_pool(name="mm", bufs=2))
            rpool = ctx.enter_context(tc.tile_pool(name="red", bufs=2))
            opool = ctx.enter_context(tc.tile_pool(name="outsb", bufs=2))
            qpool = ctx.enter_context(tc.tile_pool(name="qt", bufs=2))
            psS = ctx.enter_context(
                tc.tile_pool(name="psS", bufs=3, space="PSUM"))
            psF = ctx.enter_context(
                tc.tile_pool(name="psF", bufs=2, space="PSUM"))
            psI = ctx.enter_context(
                tc.tile_pool(name="psI", bufs=1, space="PSUM"))

            nc.gpsimd.load_library(library_config.attnmlp)

            def ld(dst, src):
                nc.sync.dma_start(dst, src)

            ident = cpool.tile([128, 128], F32, name="ident")
            ld(ident[:], ident_d)
            sel8 = cpool.tile([96, 8], F32, name="sel8")
            ld(sel8[:], sel8_d)
            rep8 = cpool.tile([8, 96], F32, name="rep8")
            ld(rep8[:], rep8_d)
            selr = cpool.tile([4, 4 * 96], F32, name="selr")
            ld(selr[:], selr_d)
            ones1 = cpool.tile([1, Q384], F32, name="ones1")
            ld(ones1[:], ones1_d)
            cv = cpool.tile([96, 8], F32, name="cv")
            ld(cv[:], cv_d)
            woff_sb = cpool.tile([128, 2, 192], F32, name="woff_sb")
            ld(woff_sb[:], woff.rearrange("(kt p) m -> p kt m", p=128))
            wattn_sb = cpool.tile([128, 2, 96], F32, name="wattn_sb")
            ld(wattn_sb[:], wattn.rearrange("(kt p) m -> p kt m", p=128))
            boff_sb = cpool.tile([1, 192], F32, name="boff_sb")
            ld(boff_sb[:], boff)
            battn_sb = cpool.tile([1, 96], F32, name="battn_sb")
            ld(battn_sb[:], battn)

            def cvs(k):
                return cv[:, k:k + 1]

            def fs(nm):
                return fpool.tile([96, Q384], F32, name=nm, tag="fs")

            qi = 0  # SWDGE queue rotation

            for b4 in range(BPC * repeat):
                b = b4 % BPC
                if "front" not in stages:
                    continue
                # ==== 1. frontend ========================================
                qsb = qpool.tile([128, NQB, DM], F32, name="qsb", tag="qsb")
                nc.vector.memset(qsb[:, 2, :], 0.0)
                nc.sync.dma_start(
                    qsb[:, 0:2, :],
                    query[b][0:256].rearrange("(s r) c -> r s c", r=128))
                nc.sync.dma_start(qsb[0:44, 2, :], query[b][256:300, :])
                refsb = qpool.tile([128, NQB, 4], F32, name="refsb",
                                   tag="refsb")
                nc.vector.memset(refsb[:, 2, :], 0.0)
                nc.sync.dma_start(
                    refsb[:, 0:2, :],
                    refp[b][0:256].rearrange("(s r) c -> r s c", r=128))
                nc.sync.dma_start(refsb[0:44, 2, :], refp[b][256:300, :])

                qT = [qpool.tile([128, Q384], F32, name=f"qT{kt}",
                                 tag=f"qT{kt}") for kt in range(2)]
                for slot in range(NQB):
                    for kt in range(2):
                        pt = psS.tile([128, 128], F32, name="pss", tag="pss")
                        nc.tensor.transpose(
                            pt[:], qsb[:, slot, kt * 128:(kt + 1) * 128],
                            ident[:])
                        nc.scalar.copy(qT[kt][:, slot * 128:(slot + 1) * 128],
                                       pt[:])
                refT = fs("refT")
                for slot in range(NQB):
                    pt = psS.tile([128, 128], F32, name="pss", tag="pss")
                    nc.tensor.transpose(pt[0:4, :], refsb[:, slot, :],
                                        ident[:])
                    nc.scalar.copy(refT[0:4, slot * 128:(slot + 1) * 128],
                                   pt[0:4, :])

                refb = []
                for m in range(4):
                    ps = psF.tile([96, Q384], F32, name="psf", tag="psf")
                    nc.tensor.matmul(ps[:], selr[:, m * 96:(m + 1) * 96],
                                     refT[0:4, :])
                    sb = fs(f"refb{m}")
                    nc.scalar.copy(sb[:], ps[:])
                    refb.append(sb)
                rxw, ryw, rwc, rhc = refb

                def head_mm(w_sb, bias_sb, cols):
                    ps = psF.tile([96, Q384], F32, name="psf", tag="psf")
                    nc.tensor.matmul(ps[:], w_sb[:, 0, cols], qT[0][:],
                                     start=True, stop=False)
                    nc.tensor.matmul(ps[:], w_sb[:, 1, cols], qT[1][:],
                                     start=False, stop=False)
                    nc.tensor.matmul(ps[:], bias_sb[:, cols], ones1[:],
                                     start=False, stop=True)
                    return ps

                logit_ps = head_mm(wattn_sb, battn_sb, slice(0, 96))
                expT = fs("expT")
                nc.scalar.activation(expT[:], logit_ps[:], EXP)
                s_ps = psF.tile([96, Q384], F32, name="psf", tag="psf")
                nc.tensor.matmul(s_ps[0:8, :], sel8[:], expT[:])
                rsum = fs("rsum")
                nc.vector.reciprocal(rsum[0:8, :], s_ps[0:8, :])
                rb_ps = psF.tile([96, Q384], F32, name="psf", tag="psf")
                nc.tensor.matmul(rb_ps[:], rep8[:], rsum[0:8, :])
                attnT = fs("attnT")
                nc.vector.tensor_tensor(attnT[:], expT[:], rb_ps[:], MUL)

                offx_ps = head_mm(woff_sb, boff_sb, slice(0, 192, 2))
                offx = fs("offx")
                nc.scalar.copy(offx[:], offx_ps[:])
                offy_ps = head_mm(woff_sb, boff_sb, slice(1, 192, 2))
                offy = fs("offy")
                nc.scalar.copy(offy[:], offy_ps[:])

                # --- x: pair-slot weights wA, wB; clipped base axb -------
                t1x = fs("t1x")
                nc.vector.tensor_tensor(t1x[:], offx[:], rwc[:], MUL)
                ixp = fs("ixp")
                nc.vector.scalar_tensor_tensor(
                    ixp[:], t1x[:], CAST_BIAS, rxw[:], ADD, ADD)
                fi = i32pool.tile([96, Q384], I32, name="fi", tag="fi")
                nc.vector.tensor_copy(fi[:], ixp[:])
                fxp = fs("fxp")
                nc.scalar.copy(fxp[:], fi[:])
                a0x = fs("a0x")
                nc.vector.tensor_scalar(a0x[:], fxp[:], SHIFT, cvs(0),
                                        MAX, MIN)
                a1x = fs("a1x")
                nc.vector.tensor_scalar(a1x[:], fxp[:], SHIFT - 1.0, cvs(1),
                                        MAX, MIN)
                axb = fs("axb")
                nc.vector.tensor_scalar(axb[:], fxp[:], SHIFT, cvs(1),
                                        MAX, MIN)
                v0x = fs("v0x")
                nc.vector.tensor_tensor(v0x[:], a0x[:], fxp[:], EQ)
                v1x = fs("v1x")
                nc.vector.tensor_tensor(v1x[:], a1x[:], fxp[:], EQ)
                fx = fs("fx")
                nc.vector.scalar_tensor_tensor(
                    fx[:], ixp[:], SHIFT - CAST_BIAS - 0.5, fxp[:], ADD, SUB)
                omfx = fs("omfx")
                nc.vector.tensor_scalar(omfx[:], fx[:], -1.0, 1.0, MUL, ADD)
                wx0 = fs("wx0")
                nc.vector.tensor_tensor(wx0[:], omfx[:], v0x[:], MUL)
                wx1 = fs("wx1")
                nc.vector.tensor_tensor(wx1[:], fx[:], v1x[:], MUL)
                eqN = fs("eqN")
                nc.vector.tensor_tensor(eqN[:], fxp[:], axb[:], EQ)
                eqm1 = fs("eqm1")
                nc.vector.tensor_scalar(eqm1[:], fxp[:], SHIFT - 1.0, None,
                                        EQ)
                eqW1 = fs("eqW1")
                nc.vector.tensor_scalar(eqW1[:], fxp[:], cvs(0), None, EQ)
                u0 = fs("u0")
                nc.vector.tensor_tensor(u0[:], eqN[:], wx0[:], MUL)
                u1 = fs("u1")
                nc.vector.tensor_tensor(u1[:], eqm1[:], wx1[:], MUL)
                wA = fs("wA")
                nc.vector.tensor_tensor(wA[:], u0[:], u1[:], ADD)
                u2 = fs("u2")
                nc.vector.tensor_tensor(u2[:], eqN[:], wx1[:], MUL)
                u3 = fs("u3")
                nc.vector.tensor_tensor(u3[:], eqW1[:], wx0[:], MUL)
                wB = fs("wB")
                nc.vector.tensor_tensor(wB[:], u2[:], u3[:], ADD)

                # --- y: pair-slot weights wyA, wyB; base ayb -------------
                t1y = fs("t1y")
                nc.vector.tensor_tensor(t1y[:], offy[:], rhc[:], MUL)
                iyp = fs("iyp")
                nc.vector.scalar_tensor_tensor(
                    iyp[:], t1y[:], CAST_BIAS, ryw[:], ADD, ADD)
                fiy = i32pool.tile([96, Q384], I32, name="fiy", tag="fiy")
                nc.vector.tensor_copy(fiy[:], iyp[:])
                fyp = fs("fyp")
                nc.scalar.copy(fyp[:], fiy[:])
                a0y = fs("a0y")
                nc.vector.tensor_scalar(a0y[:], fyp[:], SHIFT, cvs(0),
                                        MAX, MIN)
                a1y = fs("a1y")
                nc.vector.tensor_scalar(a1y[:], fyp[:], SHIFT - 1.0, cvs(1),
                                        MAX, MIN)
                ayb = fs("ayb")
                nc.vector.tensor_scalar(ayb[:], fyp[:], SHIFT, cvs(1),
                                        MAX, MIN)
                v0y = fs("v0y")
                nc.vector.tensor_tensor(v0y[:], a0y[:], fyp[:], EQ)
                v1y = fs("v1y")
                nc.vector.tensor_tensor(v1y[:], a1y[:], fyp[:], EQ)
                fy = fs("fy")
                nc.vector.scalar_tensor_tensor(
                    fy[:], iyp[:], SHIFT - CAST_BIAS - 0.5, fyp[:], ADD, SUB)
                omfy = fs("omfy")
                nc.vector.tensor_scalar(omfy[:], fy[:], -1.0, 1.0, MUL, ADD)
                wy0 = fs("wy0")
                nc.vector.tensor_tensor(wy0[:], omfy[:], v0y[:], MUL)
                wy1 = fs("wy1")
                nc.vector.tensor_tensor(wy1[:], fy[:], v1y[:], MUL)
                eqNy = fs("eqNy")
                nc.vector.tensor_tensor(eqNy[:], fyp[:], ayb[:], EQ)
                eqm1y = fs("eqm1y")
                nc.vector.tensor_scalar(eqm1y[:], fyp[:], SHIFT - 1.0, None,
                                        EQ)
                eqW1y = fs("eqW1y")
                nc.vector.tensor_scalar(eqW1y[:], fyp[:], cvs(0), None, EQ)
                uy0 = fs("uy0")
                nc.vector.tensor_tensor(uy0[:], eqNy[:], wy0[:], MUL)
                uy1 = fs("uy1")
                nc.vector.tensor_tensor(uy1[:], eqm1y[:], wy1[:], MUL)
                wyA = fs("wyA")
                nc.vector.tensor_tensor(wyA[:], uy0[:], uy1[:], ADD)
                uy2 = fs("uy2")
                nc.vector.tensor_tensor(uy2[:], eqNy[:], wy1[:], MUL)
                uy3 = fs("uy3")
                nc.vector.tensor_tensor(uy3[:], eqW1y[:], wy0[:], MUL)
                wyB = fs("wyB")
                nc.vector.tensor_tensor(wyB[:], uy2[:], uy3[:], ADD)
                wyAa = fs("wyAa")
                nc.vector.tensor_tensor(wyAa[:], wyA[:], attnT[:], MUL)
                wyBa = fs("wyBa")
                nc.vector.tensor_tensor(wyBa[:], wyB[:], attnT[:], MUL)

                # --- weight products per patch corner (ys, xs) -----------
                w4 = []
                for (wya, wxs) in ((wyAa, wA), (wyAa, wB),
                                   (wyBa, wA), (wyBa, wB)):
                    pr = fs("w4")
                    nc.vector.tensor_tensor(pr[:], wya[:], wxs[:], MUL)
                    w4.append(pr)

                # --- patch index: idx = par*NPAIR + floor(lin2/2) --------
                ybw = fs("ybw")
                nc.vector.tensor_scalar(ybw[:], ayb[:], cvs(2), None, MUL)
                lin2 = fs("lin2")
                nc.vector.scalar_tensor_tensor(lin2[:], axb[:], cvs(3),
                                               ybw[:], ADD, ADD)
                mh = fs("mh")
                nc.vector.tensor_scalar(mh[:], lin2[:], 0.5, -0.25, MUL, ADD)
                mi = i32pool.tile([96, Q384], I32, name="mi", tag="mi")
                nc.vector.tensor_copy(mi[:], mh[:])
                mf = fs("mf")
                nc.scalar.copy(mf[:], mi[:])
                par = fs("par")
                nc.vector.scalar_tensor_tensor(par[:], mf[:], -2.0, lin2[:],
                                               MUL, ADD)
                idxf = fs("idxf")
                nc.vector.scalar_tensor_tensor(idxf[:], par[:], float(NPAIR),
                                               mf[:], MUL, ADD)

                osb = opool.tile([128, NQB, DM], F32, name="osb", tag="osb")

                for qb in range(NQB):
                    cols = slice(qb * 128, (qb + 1) * 128)
                    # --- weights to q-partitions: Wb [128, 384] bf16 -----
                    # free layout (hp4, pt3, h2 2, p4 4, ys2, xs2)
                    Wb = wpool.tile([128, 384], BF16, name="Wb", tag="Wb")
                    for comp in range(4):
                        ys, xs = comp // 2, comp % 2
                        ptw = psS.tile([128, 128], F32, name="pss",
                                       tag="pss")
                        nc.tensor.transpose(ptw[:, 0:96], w4[comp][:, cols],
                                            ident[0:96, 0:96])
                        for h2 in range(2):
                            nc.scalar.copy(
                                Wb[:].rearrange(
                                    "r (hp pt h2 p4 ys xs) -> "
                                    "r hp pt h2 p4 ys xs",
                                    hp=4, pt=3, h2=2, p4=4,
                                    ys=2)[:, :, :, h2, :, ys, xs],
                                ptw[:, 0:96].rearrange(
                                    "r (h pt p4) -> r h pt p4",
                                    h=8, pt=3)[:, h2::2, :, :])

                    if "gather" not in stages:
                        continue

                    # --- idx to 16-part wrap: idxq [128, 768] i16 --------
                    psx = psI.tile([16, 8, 128], F32, name="psx", tag="psx")
                    for g8 in range(8):
                        nc.tensor.transpose(
                            psx[0:16, g8, 0:96],
                            idxf[:, qb * 128 + g8 * 16:
                                 qb * 128 + g8 * 16 + 16],
                            ident[0:96, 0:96])
                    idxq = ipool.tile([128, 768], I16, name="idxq",
                                      tag="idxq")
                    for hp in range(4):
                        for h2 in range(2):
                            h = hp * 2 + h2
                            nc.scalar.copy(
                                idxq[0:16].rearrange(
                                    "r (hp pt h2 p4 g8) -> "
                                    "r hp pt h2 p4 g8",
                                    hp=4, pt=3, h2=2,
                                    p4=4)[:, hp, :, h2, :, :],
                                psx[0:16, :, h * 12:h * 12 + 12].rearrange(
                                    "r g8 (pt p4) -> r pt p4 g8", pt=3))
                    # replicate idx to all 128 partitions (log doubling)
                    nc.sync.dma_start(idxq[16:32, :], idxq[0:16, :])
                    nc.sync.dma_start(idxq[32:64, :], idxq[0:32, :])
                    nc.sync.dma_start(idxq[64:128, :], idxq[0:64, :])

                    # --- 12 dma_gather calls -----------------------------
                    G = gpool.tile([128, 96, 128], BF16, name="G", tag="G")
                    for hp in range(4):
                        src = patches[b][hp].rearrange(
                            "h2 par m e -> (h2 par m) e")
                        for pt in range(3):
                            nc.gpsimd.dma_gather(
                                G[:, (hp * 3 + pt) * 8:
                                  (hp * 3 + pt) * 8 + 8, :],
                                src,
                                idxq[:, (hp * 3 + pt) * 64:
                                     (hp * 3 + pt) * 64 + 64],
                                num_idxs=1024, num_idxs_reg=1024,
                                elem_size=128, queue_num=qi % 4)
                            qi += 1
                    # G free layout: (hp, pt, h2, p4) x (d32, ys2, xs2)

                    if "reduce" not in stages:
                        continue

                    # --- MAC: M = G * broadcast(Wb) ----------------------
                    M = mpool.tile([128, 96, 32, 4], BF16, name="M", tag="M")
                    Gv = G[:].rearrange("r k (d s) -> r k d s", d=32)
                    Wv = Wb[:].rearrange("r (k s) -> r k s", s=4)
                    Wvb = Wv.unsqueeze(2).broadcast_to([128, 96, 32, 4])
                    nc.vector.tensor_tensor(M[:], Gv, Wvb, MUL)

                    # --- reduce over (ys, xs, p) -------------------------
                    # r1: fold ys (last-dim pairs [0:2]+[2:4])
                    R1 = rpool.tile([128, 96, 32, 2], BF16, name="R1",
                                    tag="R1")
                    nc.vector.tensor_tensor(R1[:], M[:, :, :, 0:2],
                                            M[:, :, :, 2:4], ADD)
                    # r1b: fold p4 pairs {0,1}+{2,3}
                    R1v = R1[:].rearrange(
                        "r (hpt p4) d s -> r hpt p4 (d s)", p4=4)
                    R1b = rpool.tile([128, 24, 2, 64], BF16, name="R1b",
                                     tag="R1b")
                    nc.vector.tensor_tensor(R1b[:], R1v[:, :, 0:2, :],
                                            R1v[:, :, 2:4, :], ADD)
                    # r1c: fold p4 remainder
                    R1c = rpool.tile([128, 24, 64], BF16, name="R1c",
                                     tag="R1c")
                    nc.vector.tensor_tensor(R1c[:], R1b[:, :, 0, :],
                                            R1b[:, :, 1, :], ADD)
                    # r2: fold xs (stride-2) -> f32
                    R2 = rpool.tile([128, 24, 32], F32, name="R2", tag="R2")
                    R1cv = R1c[:].rearrange("r k (d s) -> r k d s", s=2)
                    nc.vector.tensor_tensor(R2[:], R1cv[:, :, :, 0],
                                            R1cv[:, :, :, 1], ADD)
                    # fold pt (3): out osb[:, qb, :]
                    R2v = R2[:].rearrange("r (hp pt h2) d -> r hp pt (h2 d)",
                                          hp=4, pt=3)
                    T1 = rpool.tile([128, 4, 64], F32, name="T1", tag="T1")
                    nc.vector.tensor_tensor(T1[:], R2v[:, :, 0, :],
                                            R2v[:, :, 1, :], ADD)
                    nc.vector.tensor_tensor(
                        osb[:, qb, :].rearrange("r (hp e) -> r hp e", hp=4),
                        T1[:], R2v[:, :, 2, :], ADD)

                # ==== output =============================================
                if "reduce" in stages:
                    nc.sync.dma_start(
                        out_d[b][0:256].rearrange("(s r) c -> r s c", r=128),
                        osb[:, 0:2, :])
                    nc.sync.dma_start(out_d[b][256:300, :], osb[0:44, 2, :])
    return nc


_CACHE = {}


def _get_nc(repeat=None, stages=ALL_STAGES):
    if repeat is None:
        repeat = REPEAT
    key = ("nc", repeat, tuple(stages))
    if key not in _CACHE:
        nc = bacc.Bacc("TRN2", target_bir_lowering=False, debug=False,
                       enable_asserts=False, num_swdge_queues=4)
        emit(nc, repeat=repeat, stages=stages)
        nc.compile()
        _CACHE[key] = nc
    return _CACHE[key]


def _in_maps(query, reference_points, input_flatten, W_off, b_off, W_attn,
             b_attn):
    query = np.asarray(query, np.float32)
    refp = np.asarray(reference_points, np.float32).reshape(B, LQ, 4)
    value = np.asarray(input_flatten, np.float32)
    consts = make_consts()
    in_maps = []
    for c in range(N_CORES):
        sl = slice(c * BPC, (c + 1) * BPC)
        in_maps.append({
            "patches": make_patches(value[sl]),
            "query": query[sl], "refp": refp[sl],
            "woff": np.asarray(W_off, np.float32),
            "wattn": np.asarray(W_attn, np.float32),
            "boff": np.asarray(b_off, np.float32).reshape(1, 192),
            "battn": np.asarray(b_attn, np.float32).reshape(1, 96),
            **{k: v for k, v in consts.items()},
        })
    return in_maps


def kernel(query, reference_points, input_flatten, W_off, b_off, W_attn,
           b_attn):
    in_maps = _in_maps(query, reference_points, input_flatten, W_off, b_off,
                       W_attn, b_attn)
    res = run_bass_kernel_spmd(_get_nc(), in_maps,
                               core_ids=list(range(N_CORES)))
    out = np.concatenate([r["out"] for r in res.results], axis=0)
    return out.astype(np.float32)


if __name__ == "__main__":
    import sys
    sys.path.insert(0, "/root/problem")
    import reference
    inputs = {k: np.asarray(v) for k, v in reference.setup_inputs().items()}
    got = kernel(**inputs)
    exp = np.asarray(reference.reference(**inputs))
    err = np.abs(got - exp).max() / np.abs(exp).max()
    print("Relative error:", err)
